# revision 24
# baseline (speedup 1.0000x reference)
"""Multi-head attention (B=4, S=2048, D=2048, H=16, dk=128) on 8 TRN2 NeuronCores.

v3 sharding: core c handles batch b = c // 2 and the 8 heads hh = c % 2
(heads hh*8 .. hh*8+8) over the FULL sequence.  No redundant K/V projections.
After attention, the two cores of a batch pair exchange attention outputs for
the query rows they don't own via a pairwise AllGather collective, then each
core runs the output projection for its own 1024 query rows.

SPMD uniformity: the program is identical on all cores.  Divergence lives in
host-prepared data only:
  - x^T arrives with the core's OWNED query columns rolled to the front, so
    "my rows" are always local columns 0..1023 and the slice sent to the peer
    is always local columns 1024..2047.  (The rolled orders of the two pair
    members are mutually inverse, so the sent slice lands in exactly the
    receiver's local row order.)
  - the AllGather output has slot s = pair-member s's contribution; which slot
    holds the PEER's heads depends on parity, so the out-projection multiplies
    BOTH slots and the host zeroes the w_o rows of the useless slot
    (24 accumulating matmuls per PSUM tile: 8 own-head + 16 gathered slots).

All matmul operands bfloat16 (PSUM fp32): bf16 halves the PE's self-loading
LDWEIGHTS time (~112ns/tile vs fp32r's ~224ns), which hides fully under N=512
matmul compute (213ns) -> ~1 col/cycle.

Layout (zero on-chip transposes) as before: Q^T/K^T transposed via w-col-block
lhsT; V natural; scores transposed [sk, sq]; P^T = exp(scores^T/sqrt(dk)) with
no max subtraction; row sums via ones-matmul; attn_out^T = V_h^T @ P^T with
1/rowsum folded in via a PE-broadcast reciprocal.
"""

import os
import sys

import numpy as np

for _p in ("/opt/trn_rl_repo", "/root/.axon_site/_ro/trn_rl_repo"):
    if os.path.isdir(_p) and _p not in sys.path:
        sys.path.insert(0, _p)

P = 128

_CACHE = {}

PAIRS = [[0, 1], [2, 3], [4, 5], [6, 7]]


def _bf16(a):
    import ml_dtypes

    return np.ascontiguousarray(a, dtype=np.float32).astype(ml_dtypes.bfloat16)


def build_nc(D=2048, S=2048):
    """Build the single-core Bass program (SPMD: identical on all cores)."""
    from contextlib import ExitStack

    import concourse.tile as tile
    from concourse import bacc, mybir

    F32 = mybir.dt.float32
    BF16 = mybir.dt.bfloat16
    Exp = mybir.ActivationFunctionType.Exp

    HQ = 8              # local heads per core
    DT = D // P         # d-model tiles (16)
    ST = S // P         # key tiles (16)
    SQ = S              # attention runs over ALL queries
    SQO = S // 2        # owned query rows (out-projection)
    NSKC = S // 512     # K^T / Q^T projection free-dim chunks (4)
    VC = 512            # w_v streaming chunk width
    NVC = (HQ * P) // VC    # 2 chunks cover this core's 1024 V columns
    OC = 512            # w_o streaming chunk width
    NOC = D // OC
    NWO = HQ + HQ       # 16 lhsT slots in the out-projection (own + peer)
    scale = float(1.0 / np.sqrt(128.0))

    nc = bacc.Bacc("TRN2", target_bir_lowering=False, debug=False,
                   num_devices=8)

    xt_d = nc.dram_tensor("xt", [D, S], BF16, kind="ExternalInput").ap()
    ones_d = nc.dram_tensor("ones", [P, P], BF16, kind="ExternalInput").ap()
    wq_d = nc.dram_tensor("wq", [HQ, D, P], BF16, kind="ExternalInput").ap()
    wk_d = nc.dram_tensor("wk", [HQ, D, P], BF16, kind="ExternalInput").ap()
    wv_d = nc.dram_tensor("wv", [NVC, D, VC], BF16, kind="ExternalInput").ap()
    wo_d = nc.dram_tensor("wo", [NOC, NWO * P, OC], BF16,
                          kind="ExternalInput").ap()
    out_d = nc.dram_tensor("out", [SQO, D], F32, kind="ExternalOutput").ap()

    mm = nc.tensor.matmul

    # per-core gather indices: select the PEER's slot out of the AllGather
    # output (host supplies 1 - parity, so the program stays SPMD-uniform)
    gidx_d = nc.dram_tensor("gidx", [P, 8], mybir.dt.int16,
                            kind="ExternalInput").ap()

    with tile.TileContext(nc) as tc, \
            nc.allow_low_precision(reason="bf16 matmul operands"):
        with ExitStack() as octx:
            const = octx.enter_context(tc.tile_pool(name="const", bufs=1))
            ones_sb = const.tile([P, P], BF16)
            gidx_sb = const.tile([P, 8], mybir.dt.int16)

            # DRAM bounce buffers for the pairwise attention-output exchange
            dram = octx.enter_context(
                tc.tile_pool(name="agd", bufs=1, space="DRAM"))
            ag_in = [dram.tile([P, SQO], BF16, name=f"agi{h}")
                     for h in range(HQ)]
            ag_out = [dram.tile([2, P, SQO], BF16, name=f"ago{h}")
                      for h in range(HQ)]

            # persistent SBUF residents; xt/v die before the out-projection
            # (opened LAST so releasing them keeps pool stack order)
            ao_pool = octx.enter_context(tc.tile_pool(name="aop", bufs=HQ))
            agr_pool = octx.enter_context(tc.tile_pool(name="agrp", bufs=HQ))
            mainctx = octx.enter_context(ExitStack())
            xt_pool = mainctx.enter_context(tc.tile_pool(name="xtp", bufs=DT))
            v_pool = mainctx.enter_context(tc.tile_pool(name="vp", bufs=ST))

            xt_sb = [xt_pool.tile([P, S], BF16, name=f"xts{dt}", tag="xt")
                     for dt in range(DT)]
            v_sb = [v_pool.tile([P, HQ * P], BF16, name=f"vs{t}", tag="v")
                    for t in range(ST)]
            ao_sb = [ao_pool.tile([P, SQ], BF16, name=f"ao{h}", tag="ao")
                     for h in range(HQ)]
            # peer-slot receive buffers (dma_gather output is [128, 1, n])
            agr_sb = [agr_pool.tile([P, 1, SQO], BF16, name=f"agr{h}",
                                    tag="agr")
                      for h in range(HQ)]

            # ---------------- Phase V: V = x @ w_v (natural layout) ----------------
            # dt-OUTER loop order so the first matmuls only need xt tile 0 +
            # the first w_v chunk -> PE starts while x^T is still streaming in.
            with ExitStack() as ctx:
                wvp = ctx.enter_context(tc.tile_pool(name="wvp", bufs=1))
                psV = ctx.enter_context(
                    tc.tile_pool(name="psV", bufs=1, space="PSUM"))

                # DMA issue order: xt[0], ones, wv[0] (first d-tiles ahead),
                # xt[1..], wv[1] — the first matmul only needs xt[0] + the
                # head of wv[0], and wv[1] isn't needed for ~60us.
                nc.sync.dma_start(out=xt_sb[0][:], in_=xt_d[0:P, :])
                nc.sync.dma_start(out=ones_sb[:], in_=ones_d[:])
                nc.sync.dma_start(out=gidx_sb[:], in_=gidx_d[:])
                wvb = [wvp.tile([P, DT, VC], BF16, name=f"wvb{jb}", tag="wv")
                       for jb in range(NVC)]
                ap0 = wv_d[0].rearrange("(t p) n -> p t n", p=P)
                nc.sync.dma_start(out=wvb[0][:, 0:2, :], in_=ap0[:, 0:2, :])
                nc.sync.dma_start(out=wvb[0][:, 2:, :], in_=ap0[:, 2:, :])
                for dt in range(1, DT):
                    nc.sync.dma_start(
                        out=xt_sb[dt][:], in_=xt_d[dt * P:(dt + 1) * P, :])
                nc.sync.dma_start(
                    out=wvb[1][:], in_=wv_d[1].rearrange("(t p) n -> p t n", p=P))

                # 6 PSUM banks max so the K/Q-projection pool's 2 banks stay
                # free -> no drain stall at the phase transition
                for jb in range(NVC):
                    for lo, hi in ((0, 6), (6, 12), (12, 16)):
                        psv = [psV.tile([P, VC], F32, name=f"psv{k}",
                                        tag=f"psv{k}", bufs=1)
                               for k in range(hi - lo)]
                        for dt in range(DT):
                            for k in range(hi - lo):
                                kt = lo + k
                                mm(psv[k][:], xt_sb[dt][:, kt * P:(kt + 1) * P],
                                   wvb[jb][:, dt, :],
                                   start=(dt == 0), stop=(dt == DT - 1))
                        for k in range(hi - lo):
                            kt = lo + k
                            nc.vector.tensor_copy(
                                v_sb[kt][:, jb * VC:(jb + 1) * VC], psv[k][:])

            # ------ Fused: per-head K/Q projection (SBUF-resident) + attention ------
            with ExitStack() as ctx:
                wqk = ctx.enter_context(tc.tile_pool(name="wqk", bufs=1))
                iok = ctx.enter_context(tc.tile_pool(name="iok", bufs=1))
                pt_pool = ctx.enter_context(tc.tile_pool(name="ptp", bufs=1))
                sm2 = ctx.enter_context(tc.tile_pool(name="sm2", bufs=1))
                ps_kq = ctx.enter_context(
                    tc.tile_pool(name="pskq", bufs=2, space="PSUM"))
                ps_pt = ctx.enter_context(
                    tc.tile_pool(name="pspt", bufs=2, space="PSUM"))
                ps_ov = ctx.enter_context(
                    tc.tile_pool(name="psov", bufs=2, space="PSUM"))
                ps_sm = ctx.enter_context(
                    tc.tile_pool(name="pssm", bufs=2, space="PSUM"))

                k2s, q2s = {}, {}

                def proj_t(h, w_d, out_tag):
                    """Transposed projection for head h: [P, S] = w_h^T @ x^T."""
                    wb = wqk.tile([P, DT, P], BF16, name="wb", tag="w", bufs=2)
                    nc.sync.dma_start(
                        out=wb[:], in_=w_d[h].rearrange("(t p) n -> p t n", p=P))
                    o2 = iok.tile([P, S], BF16, name=out_tag, tag=out_tag,
                                  bufs=2)
                    for g in range(NSKC // 2):
                        ps = [ps_kq.tile([P, 512], F32, name=f"ps{c}",
                                         tag="ps", bufs=2)
                              for c in range(2)]
                        for dt in range(DT):
                            for c in range(2):
                                sk = (2 * g + c) * 512
                                mm(ps[c][:], wb[:, dt, :],
                                   xt_sb[dt][:, sk:sk + 512],
                                   start=(dt == 0), stop=(dt == DT - 1))
                        for c in range(2):
                            sk = (2 * g + c) * 512
                            nc.vector.tensor_copy(o2[:, sk:sk + 512], ps[c][:])
                    return o2

                def emit_kq_proj(h):
                    k2s[h] = proj_t(h, wk_d, "k2")
                    q2s[h] = proj_t(h, wq_d, "q2")

                LEAD = 2

                def emit_attention(h):
                    # peer-half chunks (2, 3) first so the exchange for this
                    # head can overlap the own-half chunks (0, 1)
                    k2, q2 = k2s[h], q2s[h]
                    for sqc in (2, 3, 0, 1):
                        pso = ps_ov.tile([P, 512], F32, name="pso")
                        psb = ps_sm.tile([P, 512], F32, name="psb")
                        ptts = [None] * ST
                        for t in range(ST + LEAD):
                            if t < ST:
                                pst = ps_pt.tile([P, 512], F32, name="pst")
                                mm(pst[:], k2[:, t * P:(t + 1) * P],
                                   q2[:, sqc * 512:(sqc + 1) * 512],
                                   start=True, stop=True)
                                ptt = pt_pool.tile([P, 512], BF16, name="ptt",
                                                   tag="pt", bufs=5)
                                nc.scalar.activation(ptt[:], pst[:], Exp,
                                                     scale=scale)
                                ptts[t] = ptt
                            if t >= LEAD:
                                u = t - LEAD
                                mm(psb[:], ones_sb[:], ptts[u][:],
                                   start=(u == 0), stop=(u == ST - 1))
                                mm(pso[:], v_sb[u][:, h * P:(h + 1) * P],
                                   ptts[u][:],
                                   start=(u == 0), stop=(u == ST - 1))
                        rbc = sm2.tile([P, 512], F32, name="rbc", tag="rbc",
                                       bufs=2)
                        nc.vector.reciprocal(rbc[:], psb[:])
                        nc.vector.tensor_mul(
                            ao_sb[h][:, sqc * 512:(sqc + 1) * 512],
                            pso[:], rbc[:])
                        if sqc == 3:
                            emit_exchange(h)

                def emit_exchange(h):
                    # send my aoT for the PEER's rows (local cols SQO..), pair-
                    # AllGather, then gather ONLY the peer's slot back using
                    # the host-provided per-core index rows.  All on the
                    # gpsimd queue so the three steps stay ordered.
                    nc.gpsimd.dma_start(out=ag_in[h][:],
                                        in_=ao_sb[h][:, SQO:SQ])
                    nc.gpsimd.collective_compute(
                        "AllGather",
                        mybir.AluOpType.bypass,
                        replica_groups=PAIRS,
                        ins=[ag_in[h][:]],
                        outs=[ag_out[h][:]],
                    )
                    nc.gpsimd.dma_gather(
                        out_ap=agr_sb[h][:],
                        in_ap=ag_out[h].rearrange("s p n -> (s p) n"),
                        idxs_ap=gidx_sb[:],
                        num_idxs=P,
                        num_idxs_reg=P,
                        elem_size=SQO,
                    )

                emit_kq_proj(0)
                for h in range(HQ):
                    if h + 1 < HQ:
                        emit_kq_proj(h + 1)
                    emit_attention(h)

            # close xt/v pools before the out-projection scope
            mainctx.close()

            # ---------------- Out-projection (own 1024 rows) ----------------
            # 16 lhsT slots per PSUM tile: 8 own heads (SBUF) + 8 received
            # peer heads.  The host orders w_o rows [own heads | peer heads].
            with ExitStack() as ctx:
                wo3 = ctx.enter_context(tc.tile_pool(name="wo3", bufs=1))
                ev3 = ctx.enter_context(tc.tile_pool(name="ev3", bufs=1))
                ps3p = ctx.enter_context(
                    tc.tile_pool(name="ps3p", bufs=4, space="PSUM"))

                for oc in range(NOC):
                    wob = wo3.tile([P, NWO, OC], BF16, name="wob", tag="wo",
                                   bufs=2)
                    nc.sync.dma_start(
                        out=wob[:], in_=wo_d[oc].rearrange("(t p) n -> p t n", p=P))
                    for sqt in range(SQO // P):
                        ps3 = ps3p.tile([P, OC], F32, name="ps3")
                        sl = slice(sqt * P, (sqt + 1) * P)
                        for j in range(NWO):
                            if j < HQ:
                                lhsT = ao_sb[j][:, sl]
                            else:
                                lhsT = agr_sb[j - HQ][:, 0, sl]
                            mm(ps3[:], lhsT, wob[:, j, :],
                               start=(j == 0), stop=(j == NWO - 1))
                        oev = ev3.tile([P, OC], F32, name="oev", tag="oev",
                                       bufs=6)
                        nc.vector.tensor_copy(oev[:], ps3[:])
                        nc.sync.dma_start(
                            out=out_d[sqt * P:(sqt + 1) * P,
                                      oc * OC:(oc + 1) * OC],
                            in_=oev[:])

    nc.compile()
    return nc


def prep_inputs(x, w_q, w_k, w_v, w_o, D=2048, S=2048, n_cores=8):
    """Host-side shard + re-layout. Returns in_maps for run_bass_kernel_spmd."""
    HQ = 8
    SQO = S // 2
    NOC = D // 512
    ones = _bf16(np.ones((P, P), dtype=np.float32))
    # per head-half: w_q/w_k column blocks, w_v column slice
    wq_h, wk_h, wv_h, wo_h = [], [], [], []
    for hh in range(2):
        hsl = slice(hh * HQ * P, (hh + 1) * HQ * P)
        wq_h.append(_bf16(w_q[:, hsl].reshape(D, HQ, P).transpose(1, 0, 2)))
        wk_h.append(_bf16(w_k[:, hsl].reshape(D, HQ, P).transpose(1, 0, 2)))
        wv_h.append(_bf16(w_v[:, hsl].reshape(D, 2, 512).transpose(1, 0, 2)))
        # out-projection slots: [own 8 heads' w_o rows | peer 8 heads' rows]
        psl = slice((1 - hh) * HQ * P, (2 - hh) * HQ * P)
        wo_pad = np.concatenate([w_o[hsl, :], w_o[psl, :]], axis=0)  # [16*P, D]
        wo_h.append(_bf16(
            wo_pad.reshape(16 * P, NOC, 512).transpose(1, 0, 2)))
    # gather indices: row k of the peer's AllGather slot, wrapped so that
    # idx position k lives at [k % 16, k // 16] (replicated to 128 partitions)
    gidx_h = []
    for hh in range(2):
        k = (np.arange(8)[None, :] * 16 + np.arange(128)[:, None] % 16)
        gidx_h.append(((1 - hh) * P + k).astype(np.int16))
    in_maps = []
    for c in range(n_cores):
        b, hh = divmod(c, 2)
        xt = x[b].T  # [D, S]
        # roll this core's OWNED query columns to the front
        xt = _bf16(np.roll(xt, -hh * SQO, axis=1))
        in_maps.append({
            "xt": xt, "wq": wq_h[hh], "wk": wk_h[hh], "wv": wv_h[hh],
            "wo": wo_h[hh], "ones": ones, "gidx": gidx_h[hh],
        })
    return in_maps


def run(x, w_q, w_k, w_v, w_o, trace=False):
    from concourse.bass_utils import run_bass_kernel_spmd

    B, S, D = x.shape
    n_cores = 8
    SQO = S // 2
    key = (D, S)
    if key not in _CACHE:
        _CACHE[key] = build_nc(D=D, S=S)
    nc = _CACHE[key]
    in_maps = prep_inputs(x, w_q, w_k, w_v, w_o, D=D, S=S, n_cores=n_cores)
    res = run_bass_kernel_spmd(nc, in_maps, core_ids=list(range(n_cores)), trace=trace)
    out = np.empty((B, S, D), dtype=np.float32)
    for c in range(n_cores):
        b, hh = divmod(c, 2)
        out[b, hh * SQO:(hh + 1) * SQO, :] = res.results[c]["out"]
    return out, res


def kernel(x, w_q, w_k, w_v, w_o):
    out, _ = run(np.asarray(x), np.asarray(w_q), np.asarray(w_k),
                 np.asarray(w_v), np.asarray(w_o))
    return out
